# revision 1
# baseline (speedup 1.0000x reference)
"""DMN (Dynamic Memory Network) forward pass on 8 Trainium2 NeuronCores.

Fully data-parallel over batch (16 examples/core, no cross-core traffic).
Matmuls in bf16 with fp32 PSUM accumulation. GRU states live in a "folded
transposed" SBUF layout (128 partitions = one 128-row slice of H; free dim =
h_tile * batch + example), so the recurrent matmul's moving operand needs no
per-step transposes and pointwise ops stay wide.

kernel(**inputs) takes FULL unsharded inputs (as from reference.setup_inputs)
and returns the FULL (B*num_decode, V) fp32 output.
"""

import numpy as np
import ml_dtypes

import concourse.bacc as bacc
import concourse.mybir as mybir
import concourse.tile as tile
from concourse import bass_utils

F32 = mybir.dt.float32
BF16 = mybir.dt.bfloat16
AF = mybir.ActivationFunctionType
ALU = mybir.AluOpType

H = 512
HQ = 4            # H / 128
G3 = 3 * H
MT = 12           # gate m-tiles
V = 32000
B = 128
NF = 40
L = 12
QL = 16
EPISODES = 3
N_CORES = 8
BC = B // N_CORES
FCHUNK = 320
VBLK = 512

bf16 = ml_dtypes.bfloat16

_COMPILED = {}


class Cfg:
    def __init__(self, bc=BC, nf=NF, l=L, ql=QL, ep=EPISODES, nd=4, v=V,
                 fchunk=FCHUNK):
        self.bc, self.nf, self.l, self.ql, self.ep, self.nd, self.v = \
            bc, nf, l, ql, ep, nd, v
        self.s = bc * nf
        self.fchunk = min(fchunk, self.s)
        assert self.s % self.fchunk == 0
        self.nfc = self.s // self.fchunk
        self.vblks = [min(VBLK, v - i) for i in range(0, v, VBLK)]
        self.nv = bc * nd
        self.key = (bc, nf, l, ql, ep, nd, v, self.fchunk)


def _wt(wsb, k, m):
    """lhsT tile (128,128) of a weight sbuf tensor laid out (128, KT*G3)."""
    return wsb[:, k * G3 + m * 128:k * G3 + (m + 1) * 128]


def build(cfg: Cfg):
    nc = bacc.Bacc("TRN2", target_bir_lowering=False, debug=False,
                   num_devices=N_CORES)
    bc, nf, l, ql, ep, nd, v = cfg.bc, cfg.nf, cfg.l, cfg.ql, cfg.ep, cfg.nd, cfg.v
    s, ch, nfc, nv = cfg.s, cfg.fchunk, cfg.nfc, cfg.nv
    ntq = bc * ql

    def din(name, shape, dt=BF16):
        return nc.dram_tensor(name, list(shape), dt, kind="ExternalInput").ap()

    fx = din("fx", (s, l, H))
    qx = din("qx", (ntq, H))
    w_f_ih = din("w_f_ih", (128, HQ * G3)); w_f_hh = din("w_f_hh", (128, HQ * G3))
    w_q_ih = din("w_q_ih", (128, HQ * G3)); w_q_hh = din("w_q_hh", (128, HQ * G3))
    w_a_ih = din("w_a_ih", (128, HQ * G3)); w_a_hh = din("w_a_hh", (128, HQ * G3))
    w_m_ih = din("w_m_ih", (128, HQ * G3)); w_m_hh = din("w_m_hh", (128, HQ * G3))
    w_ans_ih = din("w_ans_ih", (128, 2 * HQ * G3))
    w_ans_hh = din("w_ans_hh", (128, HQ * G3))
    g1t = din("g1t", (128, 16 * H))
    g2t_d = din("g2t", (128, HQ))
    fct = din("fct", (128, HQ, v))
    fcb = din("fcb", (1, v))
    y0t_d = din("y0t", (128, HQ))
    gib_q_d = din("gib_q", (128, MT), F32)
    gib_a_d = din("gib_a", (128, MT), F32)
    gib_ans_d = din("gib_ans", (128, MT), F32)
    brz_f_d = din("brz_f", (128, 8), F32)
    bnih_f_d = din("bnih_f", (128, 4), F32)
    bnhh_f_d = din("bnhh_f", (128, 4), F32)
    bnhh_q_d = din("bnhh_q", (128, 4), F32)
    bnhh_a_d = din("bnhh_a", (128, 4), F32)
    bnhh_ans_d = din("bnhh_ans", (128, 4), F32)
    brz_m_d = din("brz_m", (128, 8), F32)
    bnih_m_d = din("bnih_m", (128, 4), F32)
    bnhh_m_d = din("bnhh_m", (128, 4), F32)
    gb1_d = din("gb1", (128, HQ), F32)
    gb2_d = din("gb2", (1, 1), F32)
    out_d = nc.dram_tensor("out", [nv, v], F32, kind="ExternalOutput").ap()

    with tile.TileContext(nc) as tc, tc.tile_pool(name="const", bufs=1) as cp:
        frepT = cp.tile([128, HQ * s], BF16, tag="frepT")
        qrepT = cp.tile([128, HQ * bc], BF16, tag="qrepT")
        memT = cp.tile([128, HQ * bc], BF16, tag="memT")
        ones_nv = cp.tile([1, nv], BF16, tag="ones_nv")
        nc.vector.memset(ones_nv[:], 1.0)
        ones_128 = cp.tile([1, 128], BF16, tag="ones_128")
        nc.vector.memset(ones_128[:], 1.0)

        def load(ap_d, shape, dt=F32):
            t = cp.tile(list(shape), dt, tag=ap_d.tensor.name + "_sb")
            nc.sync.dma_start(t[:], ap_d[:])
            return t

        gib_q = load(gib_q_d, (128, MT))
        gib_a = load(gib_a_d, (128, MT))
        gib_ans = load(gib_ans_d, (128, MT))
        brz_f = load(brz_f_d, (128, 8))
        bnih_f = load(bnih_f_d, (128, 4))
        bnhh_f = load(bnhh_f_d, (128, 4))
        gb1 = load(gb1_d, (128, HQ))
        gb2 = load(gb2_d, (1, 1))
        y0t = load(y0t_d, (128, HQ), BF16)
        g2t = load(g2t_d, (128, HQ), BF16)

        def bcast_cols(src, ncol, rep, tag):
            t = cp.tile([128, ncol * rep], F32, tag=tag)
            nc.vector.tensor_copy(
                t[:].rearrange("p (c r) -> p c r", c=ncol),
                src[:].to_broadcast([128, ncol, rep]))
            return t

        bnhhx_q = bcast_cols(load(bnhh_q_d, (128, 4)), 4, bc, "bnhhx_q")
        bnhhx_a = bcast_cols(load(bnhh_a_d, (128, 4)), 4, bc, "bnhhx_a")
        bnhhx_ans = bcast_cols(load(bnhh_ans_d, (128, 4)), 4, bc, "bnhhx_ans")
        brzx_m = bcast_cols(load(brz_m_d, (128, 8)), 8, bc, "brzx_m")
        bnihx_m = bcast_cols(load(bnih_m_d, (128, 4)), 4, bc, "bnihx_m")
        bnhhx_m = bcast_cols(load(bnhh_m_d, (128, 4)), 4, bc, "bnhhx_m")

        def gru_small(sp, pp, contribs, h_ap, out_ap, bn, gi_rz=None, gi_n=None,
                      bnhhx=None, brzx=None, nihc=None, bnihx=None, g_bc=None):
            """One folded GRU step for bn<=32.

            contribs: [(w_sb, src_ap, nk), ...] all feed rz; contribs[0] (the
            hh path) alone feeds the n_hh slab. nihc=(w,src,nk) optionally
            feeds a separate n_ih slab (fused-x path, biases via brzx/bnihx).
            """
            ps = pp.tile([128, 8 * bn], F32, tag="gps_rz")
            nslab = 4 if nihc is None else 8
            psn = pp.tile([128, nslab * bn], F32, tag="gps_n")
            ncon = sum(c[2] for c in contribs)
            for m in range(8):
                i = 0
                for (wsb, src, nk) in contribs:
                    for k in range(nk):
                        nc.tensor.matmul(ps[:, m * bn:(m + 1) * bn],
                                         _wt(wsb, k, m),
                                         src[:, k * bn:(k + 1) * bn],
                                         start=i == 0, stop=i == ncon - 1)
                        i += 1
            wsb0, src0, nk0 = contribs[0]
            for mi in range(4):
                for k in range(nk0):
                    nc.tensor.matmul(psn[:, mi * bn:(mi + 1) * bn],
                                     _wt(wsb0, k, 8 + mi),
                                     src0[:, k * bn:(k + 1) * bn],
                                     start=k == 0, stop=k == nk0 - 1)
            if nihc is not None:
                wsb1, src1, nk1 = nihc
                for mi in range(4):
                    for k in range(nk1):
                        nc.tensor.matmul(psn[:, (4 + mi) * bn:(5 + mi) * bn],
                                         _wt(wsb1, k, 8 + mi),
                                         src1[:, k * bn:(k + 1) * bn],
                                         start=k == 0, stop=k == nk1 - 1)

            rzpre = sp.tile([128, 8 * bn], F32, tag="rzpre")
            rz = sp.tile([128, 8 * bn], BF16, tag="rz")
            # r half first so its sigmoid overlaps the n-slab matmuls
            if gi_rz is not None:
                gi_rz8 = gi_rz
                for h0, h1 in ((0, 4), (4, 8)):
                    nc.vector.tensor_add(
                        rzpre[:, h0 * bn:h1 * bn].rearrange(
                            "p (m b) -> p m b", m=4),
                        ps[:, h0 * bn:h1 * bn].rearrange(
                            "p (m b) -> p m b", m=4), gi_rz8[:, h0:h1])
                    nc.scalar.activation(rz[:, h0 * bn:h1 * bn],
                                         rzpre[:, h0 * bn:h1 * bn], AF.Sigmoid)
            else:
                for h0, h1 in ((0, 4), (4, 8)):
                    nc.vector.tensor_add(rzpre[:, h0 * bn:h1 * bn],
                                         ps[:, h0 * bn:h1 * bn],
                                         brzx[:, h0 * bn:h1 * bn])
                    nc.scalar.activation(rz[:, h0 * bn:h1 * bn],
                                         rzpre[:, h0 * bn:h1 * bn], AF.Sigmoid)

            t0 = sp.tile([128, 4 * bn], F32, tag="t0")
            nc.vector.tensor_add(t0[:], psn[:, 0:4 * bn], bnhhx[:])
            t1 = sp.tile([128, 4 * bn], F32, tag="t1")
            nc.vector.tensor_mul(t1[:], rz[:, 0:4 * bn], t0[:])
            npre = sp.tile([128, 4 * bn], F32, tag="npre")
            if gi_n is not None:
                nc.vector.tensor_add(
                    npre[:].rearrange("p (m b) -> p m b", m=4),
                    t1[:].rearrange("p (m b) -> p m b", m=4), gi_n)
            else:
                x1 = sp.tile([128, 4 * bn], F32, tag="x1")
                nc.vector.tensor_add(x1[:], psn[:, 4 * bn:8 * bn], bnihx[:])
                nc.vector.tensor_add(npre[:], t1[:], x1[:])
            n_t = sp.tile([128, 4 * bn], BF16, tag="n_t")
            nc.scalar.activation(n_t[:], npre[:], AF.Tanh)

            w_t = sp.tile([128, 4 * bn], BF16, tag="w_t")
            nc.vector.tensor_scalar(w_t[:], rz[:, 4 * bn:8 * bn], -1.0, 1.0,
                                    ALU.mult, ALU.add)
            d_t = sp.tile([128, 4 * bn], BF16, tag="d_t")
            nc.vector.tensor_sub(d_t[:], n_t[:], h_ap)
            u_t = sp.tile([128, 4 * bn], BF16, tag="u_t")
            nc.vector.tensor_mul(u_t[:], w_t[:], d_t[:])
            if g_bc is not None:
                u2 = sp.tile([128, 4 * bn], BF16, tag="u2")
                nc.vector.tensor_mul(
                    u2[:].rearrange("p (q b) -> p b q", q=HQ),
                    u_t[:].rearrange("p (q b) -> p b q", q=HQ), g_bc)
                u_t = u2
            nc.vector.tensor_add(out_ap, h_ap, u_t[:])

        # -------- facts GRU with question GRU interleaved --------
        with tc.tile_pool(name="wf", bufs=1) as wf, \
             tc.tile_pool(name="fxp", bufs=8) as xp, \
             tc.tile_pool(name="fps", bufs=5, space="PSUM") as pp, \
             tc.tile_pool(name="qpsA", bufs=1, space="PSUM") as ppa, \
             tc.tile_pool(name="qpsB", bufs=1, space="PSUM") as ppb, \
             tc.tile_pool(name="fsp", bufs=3) as sp, \
             tc.tile_pool(name="fst", bufs=1) as stp:
            wih = wf.tile([128, HQ * G3], BF16, tag="wih")
            whh = wf.tile([128, HQ * G3], BF16, tag="whh")
            nc.sync.dma_start(wih[:], w_f_ih[:])
            nc.sync.dma_start(whh[:], w_f_hh[:])
            wihq = wf.tile([128, HQ * G3], BF16, tag="wihq")
            whhq = wf.tile([128, HQ * G3], BF16, tag="whhq")
            nc.sync.dma_start(wihq[:], w_q_ih[:])
            nc.sync.dma_start(whhq[:], w_q_hh[:])
            # question-GRU input-gate precompute (bias folded)
            qxT = stp.tile([128, HQ * ntq], BF16, tag="qxT")
            for q in range(HQ):
                nc.sync.dma_start_transpose(
                    qxT[:, q * ntq:(q + 1) * ntq], qx[:, q * 128:(q + 1) * 128])
            giq = stp.tile([128, MT * ntq], BF16, tag="giq")
            for m in range(MT):
                psm = ppa.tile([128, ntq], F32, tag="qg")
                for k in range(HQ):
                    nc.tensor.matmul(psm[:], _wt(wihq, k, m),
                                     qxT[:, k * ntq:(k + 1) * ntq],
                                     start=k == 0, stop=k == HQ - 1)
                nc.scalar.activation(giq[:, m * ntq:(m + 1) * ntq], psm[:],
                                     AF.Identity, bias=gib_q[:, m:m + 1])
            hq = stp.tile([128, HQ * bc], BF16, tag="hq")
            nc.vector.memset(hq[:], 0.0)
            gi4 = giq[:].rearrange("p (m b t) -> p m b t", m=MT, b=bc)
            qstep = [0]

            def q_step():
                t = qstep[0]
                if t >= ql:
                    return
                qstep[0] += 1
                out_ap = qrepT[:] if t == ql - 1 else hq[:]
                gru_small(sp, ppb, [(whhq, hq[:], HQ)], hq[:], out_ap, bc,
                          gi_rz=gi4[:, 0:8, :, t:t + 1].rearrange(
                              "p m b o -> p m (b o)"),
                          gi_n=gi4[:, 8:12, :, t:t + 1].rearrange(
                              "p m b o -> p m (b o)"),
                          bnhhx=bnhhx_q)

            hst = []
            for c in range(nfc):
                h = stp.tile([128, HQ * ch], BF16, tag=f"hf{c}")
                nc.vector.memset(h[:], 0.0)
                hst.append(h)
            for t in range(l):
                for c in range(nfc):
                    h = hst[c]
                    xt = xp.tile([128, HQ * ch], BF16, tag="xt")
                    for q in range(HQ):
                        nc.sync.dma_start_transpose(
                            xt[:, q * ch:(q + 1) * ch],
                            fx[c * ch:(c + 1) * ch, t:t + 1,
                               q * 128:(q + 1) * 128])
                    rz = sp.tile([128, 8 * ch], BF16, tag="rzf")
                    for m in range(8):
                        psm = pp.tile([128, ch], F32, tag="fg")
                        for k in range(HQ):
                            nc.tensor.matmul(psm[:], _wt(wih, k, m),
                                             xt[:, k * ch:(k + 1) * ch],
                                             start=k == 0, stop=False)
                        for k in range(HQ):
                            nc.tensor.matmul(psm[:], _wt(whh, k, m),
                                             h[:, k * ch:(k + 1) * ch],
                                             start=False, stop=k == HQ - 1)
                        nc.scalar.activation(rz[:, m * ch:(m + 1) * ch], psm[:],
                                             AF.Sigmoid, bias=brz_f[:, m:m + 1])
                    t_sb = sp.tile([128, 4 * ch], F32, tag="tf")
                    for j in range(4):
                        psm = pp.tile([128, ch], F32, tag="fg")
                        for k in range(HQ):
                            nc.tensor.matmul(psm[:], _wt(whh, k, 8 + j),
                                             h[:, k * ch:(k + 1) * ch],
                                             start=k == 0, stop=k == HQ - 1)
                        nc.vector.scalar_tensor_tensor(
                            t_sb[:, j * ch:(j + 1) * ch], psm[:],
                            bnhh_f[:, j:j + 1], rz[:, j * ch:(j + 1) * ch],
                            ALU.add, ALU.mult)
                    npre = sp.tile([128, 4 * ch], F32, tag="npf")
                    for j in range(4):
                        psm = pp.tile([128, ch], F32, tag="fg")
                        for k in range(HQ):
                            nc.tensor.matmul(psm[:], _wt(wih, k, 8 + j),
                                             xt[:, k * ch:(k + 1) * ch],
                                             start=k == 0, stop=k == HQ - 1)
                        nc.vector.scalar_tensor_tensor(
                            npre[:, j * ch:(j + 1) * ch], psm[:],
                            bnih_f[:, j:j + 1], t_sb[:, j * ch:(j + 1) * ch],
                            ALU.add, ALU.add)
                    n_t = sp.tile([128, 4 * ch], BF16, tag="nf")
                    nc.scalar.activation(n_t[:], npre[:], AF.Tanh)
                    w_t = sp.tile([128, 4 * ch], BF16, tag="wtf")
                    nc.vector.tensor_scalar(w_t[:], rz[:, 4 * ch:8 * ch],
                                            -1.0, 1.0, ALU.mult, ALU.add)
                    d_t = sp.tile([128, 4 * ch], BF16, tag="df")
                    nc.vector.tensor_sub(d_t[:], n_t[:], h[:])
                    u_t = sp.tile([128, 4 * ch], BF16, tag="uf")
                    nc.vector.tensor_mul(u_t[:], w_t[:], d_t[:])
                    if t == l - 1:
                        out_ap = frepT[:].rearrange(
                            "p (q sq) -> p q sq", q=HQ)[:, :, c * ch:(c + 1) * ch]
                        nc.vector.tensor_add(
                            out_ap, h[:].rearrange("p (q sq) -> p q sq", q=HQ),
                            u_t[:].rearrange("p (q sq) -> p q sq", q=HQ))
                    else:
                        nc.vector.tensor_add(h[:], h[:], u_t[:])
                    q_step()
            while qstep[0] < ql:
                q_step()

        # ---------------- episodic memory ----------------
        with tc.tile_pool(name="we", bufs=1) as we, \
             tc.tile_pool(name="epsA", bufs=3, space="PSUM") as ppa, \
             tc.tile_pool(name="epsB", bufs=2, space="PSUM") as ppb, \
             tc.tile_pool(name="eps2", bufs=1, space="PSUM") as pp2, \
             tc.tile_pool(name="esp", bufs=3) as sp, \
             tc.tile_pool(name="est", bufs=1) as stp:
            wiha = we.tile([128, HQ * G3], BF16, tag="wiha")
            whha = we.tile([128, HQ * G3], BF16, tag="whha")
            wihm = we.tile([128, HQ * G3], BF16, tag="wihm")
            whhm = we.tile([128, HQ * G3], BF16, tag="whhm")
            g1sb = we.tile([128, 16 * H], BF16, tag="g1sb")
            nc.sync.dma_start(wiha[:], w_a_ih[:])
            nc.sync.dma_start(whha[:], w_a_hh[:])
            nc.sync.dma_start(wihm[:], w_m_ih[:])
            nc.sync.dma_start(whhm[:], w_m_hh[:])
            nc.sync.dma_start(g1sb[:], g1t[:])
            nc.vector.tensor_copy(memT[:], qrepT[:])
            qexp = stp.tile([128, HQ * s], BF16, tag="qexp")
            nc.vector.tensor_copy(
                qexp[:].rearrange("p (qb f) -> p qb f", f=nf),
                qrepT[:].to_broadcast([128, HQ * bc, nf]))
            zfeat = stp.tile([128, 16 * s], BF16, tag="zfeat")
            mexp = stp.tile([128, HQ * s], BF16, tag="mexp")
            gia = stp.tile([128, MT * s], BF16, tag="gia")
            gex = stp.tile([128, s], BF16, tag="gex")
            he = stp.tile([128, HQ * bc], BF16, tag="he")
            sblk = [min(VBLK, s - i) for i in range(0, s, VBLK)]
            # episode-invariant: f*q, |f-q| zfeat halves and the attention
            # GRU's input gates gi_a = Wih_a @ frep (+bias)
            nc.vector.tensor_mul(zfeat[:, 0:HQ * s], frepT[:], qexp[:])
            t3 = sp.tile([128, HQ * s], F32, tag="zt")
            nc.vector.tensor_sub(t3[:], frepT[:], qexp[:])
            nc.scalar.activation(zfeat[:, 2 * HQ * s:3 * HQ * s], t3[:], AF.Abs)
            for m in range(MT):
                off = 0
                for nb in sblk:
                    psm = ppa.tile([128, VBLK], F32, tag="eg")
                    for k in range(HQ):
                        nc.tensor.matmul(
                            psm[:, 0:nb], _wt(wiha, k, m),
                            frepT[:, k * s + off:k * s + off + nb],
                            start=k == 0, stop=k == HQ - 1)
                    nc.scalar.activation(
                        gia[:, m * s + off:m * s + off + nb],
                        psm[:, 0:nb], AF.Identity, bias=gib_a[:, m:m + 1])
                    off += nb
            for e in range(ep):
                nc.vector.tensor_copy(
                    mexp[:].rearrange("p (qb f) -> p qb f", f=nf),
                    memT[:].to_broadcast([128, HQ * bc, nf]))
                nc.vector.tensor_mul(zfeat[:, HQ * s:2 * HQ * s], frepT[:],
                                     mexp[:])
                t4 = sp.tile([128, HQ * s], F32, tag="zt")
                nc.vector.tensor_sub(t4[:], frepT[:], mexp[:])
                nc.scalar.activation(zfeat[:, 3 * HQ * s:4 * HQ * s], t4[:],
                                     AF.Abs)
                relu = sp.tile([128, HQ * s], BF16, tag="relu")
                for m in range(HQ):
                    off = 0
                    for nb in sblk:
                        psm = ppa.tile([128, VBLK], F32, tag="eg")
                        for k in range(16):
                            nc.tensor.matmul(
                                psm[:, 0:nb],
                                g1sb[:, k * H + m * 128:k * H + (m + 1) * 128],
                                zfeat[:, k * s + off:k * s + off + nb],
                                start=k == 0, stop=k == 15)
                        nc.scalar.activation(
                            relu[:, m * s + off:m * s + off + nb],
                            psm[:, 0:nb], AF.Relu, bias=gb1[:, m:m + 1])
                        off += nb
                off = 0
                for nb in sblk:
                    psg = pp2.tile([1, VBLK], F32, tag="eg2")
                    for k in range(HQ):
                        nc.tensor.matmul(psg[0:1, 0:nb], g2t[:, k:k + 1],
                                         relu[:, k * s + off:k * s + off + nb],
                                         start=k == 0, stop=k == HQ - 1)
                    nc.scalar.activation(gex[0:1, off:off + nb], psg[0:1, 0:nb],
                                         AF.Sigmoid, bias=gb2[:])
                    off += nb
                off = 0
                for nb in sblk:
                    psb = ppa.tile([128, VBLK], F32, tag="eg")
                    nc.tensor.matmul(psb[:, 0:nb], ones_128[:],
                                     gex[0:1, off:off + nb], start=True,
                                     stop=True)
                    nc.vector.tensor_copy(gex[:, off:off + nb], psb[:, 0:nb])
                    off += nb
                nc.vector.memset(he[:], 0.0)
                gia4 = gia[:].rearrange("p (m b f) -> p m b f", m=MT, b=bc)
                gex3 = gex[:].rearrange("p (b f) -> p b f", b=bc)
                for t in range(nf):
                    gru_small(
                        sp, ppb, [(whha, he[:], HQ)], he[:], he[:], bc,
                        gi_rz=gia4[:, 0:8, :, t:t + 1].rearrange(
                            "p m b o -> p m (b o)"),
                        gi_n=gia4[:, 8:12, :, t:t + 1].rearrange(
                            "p m b o -> p m (b o)"),
                        bnhhx=bnhhx_a,
                        g_bc=gex3[:, :, t:t + 1].to_broadcast([128, bc, HQ]))
                gru_small(sp, ppb, [(whhm, memT[:], HQ), (wihm, he[:], HQ)],
                          memT[:], memT[:], bc, nihc=(wihm, he[:], HQ),
                          brzx=brzx_m, bnihx=bnihx_m, bnhhx=bnhhx_m)

        # ---------------- answer + fc/log-softmax ----------------
        with tc.tile_pool(name="wa", bufs=1) as wa, \
             tc.tile_pool(name="apsA", bufs=1, space="PSUM") as ppa, \
             tc.tile_pool(name="apsB", bufs=2, space="PSUM") as ppb, \
             tc.tile_pool(name="fcps", bufs=3, space="PSUM") as fpp, \
             tc.tile_pool(name="asp", bufs=3) as sp, \
             tc.tile_pool(name="ast", bufs=1) as stp, \
             tc.tile_pool(name="fcw", bufs=8) as fcp, tc.tile_pool(name="fco", bufs=2) as fop:
            wihans = wa.tile([128, 2 * HQ * G3], BF16, tag="wihans")
            whhans = wa.tile([128, HQ * G3], BF16, tag="whhans")
            nc.sync.dma_start(wihans[:], w_ans_ih[:])
            nc.sync.dma_start(whhans[:], w_ans_hh[:])
            ansin = stp.tile([128, 2 * HQ * bc], BF16, tag="ansin")
            nc.vector.tensor_copy(
                ansin[:, 0:HQ * bc].rearrange("p (q b) -> p q b", q=HQ),
                y0t[:].to_broadcast([128, HQ, bc]))
            nc.vector.tensor_copy(ansin[:, HQ * bc:2 * HQ * bc], qrepT[:])
            gians = stp.tile([128, MT * bc], BF16, tag="gians")
            for m in range(MT):
                psm = ppa.tile([128, bc], F32, tag="ag")
                for k in range(2 * HQ):
                    nc.tensor.matmul(psm[:], _wt(wihans, k, m),
                                     ansin[:, k * bc:(k + 1) * bc],
                                     start=k == 0, stop=k == 2 * HQ - 1)
                nc.scalar.activation(gians[:, m * bc:(m + 1) * bc], psm[:],
                                     AF.Identity, bias=gib_ans[:, m:m + 1])
            gians3 = gians[:].rearrange("p (m b) -> p m b", m=MT)
            hdecT = stp.tile([128, HQ * nv], BF16, tag="hdecT")
            hans = stp.tile([128, HQ * bc], BF16, tag="hans")
            nc.vector.tensor_copy(hans[:], memT[:])
            hd4 = hdecT[:].rearrange("p (q b dd) -> p q b dd", q=HQ, b=bc)
            for d in range(nd):
                gru_small(sp, ppb, [(whhans, hans[:], HQ)], hans[:], hans[:],
                          bc, gi_rz=gians3[:, 0:8, :], gi_n=gians3[:, 8:12, :],
                          bnhhx=bnhhx_ans)
                nc.vector.tensor_copy(
                    hd4[:, :, :, d:d + 1],
                    hans[:].rearrange("p (q b) -> p q b",
                                      q=HQ).to_broadcast([128, HQ, bc, 1]))
            logits = stp.tile([nv, v], BF16, tag="logits")
            sums = stp.tile([nv, len(cfg.vblks)], F32, tag="sums")
            off = 0
            for bi, nb in enumerate(cfg.vblks):
                wtl = fcp.tile([128, HQ * VBLK], BF16, tag="fcwt")
                nc.sync.dma_start(
                    wtl[:, 0:HQ * nb].rearrange("p (q n) -> p q n", q=HQ),
                    fct[:, :, off:off + nb])
                fcbt = fcp.tile([1, VBLK], BF16, tag="fcbt")
                nc.sync.dma_start(fcbt[0:1, 0:nb], fcb[0:1, off:off + nb])
                psm = fpp.tile([nv, VBLK], F32, tag="fps")
                for k in range(HQ):
                    nc.tensor.matmul(psm[:, 0:nb],
                                     hdecT[:, k * nv:(k + 1) * nv],
                                     wtl[:, k * nb:(k + 1) * nb],
                                     start=k == 0, stop=False)
                nc.tensor.matmul(psm[:, 0:nb], ones_nv[:], fcbt[0:1, 0:nb],
                                 start=False, stop=True)
                ex = sp.tile([nv, VBLK], BF16, tag="ex")
                nc.scalar.activation(ex[:, 0:nb], psm[:, 0:nb], AF.Exp,
                                     accum_out=sums[:, bi:bi + 1])
                nc.vector.tensor_copy(logits[:, off:off + nb], psm[:, 0:nb])
                off += nb
            ssum = stp.tile([nv, 1], F32, tag="ssum")
            nc.vector.reduce_sum(ssum[:], sums[:], axis=mybir.AxisListType.X)
            logz = stp.tile([nv, 1], F32, tag="logz")
            nc.scalar.activation(logz[:], ssum[:], AF.Ln)
            ochunk = 4000
            for o0 in range(0, v, ochunk):
                o1 = min(o0 + ochunk, v)
                outb = fop.tile([nv, ochunk], F32, tag="outb")
                nc.vector.tensor_scalar(outb[:, 0:o1 - o0], logits[:, o0:o1],
                                        logz[:], None, ALU.subtract)
                nc.sync.dma_start(out_d[:, o0:o1], outb[:, 0:o1 - o0])
    nc.compile()
    return nc


def host_prep(inputs, cfg: Cfg):
    bc, nf, l, ql, nd, v = cfg.bc, cfg.nf, cfg.l, cfg.ql, cfg.nd, cfg.v
    emb = np.asarray(inputs["emb"], np.float32).copy()
    emb[0] = 0.0
    facts = np.asarray(inputs["facts"])
    questions = np.asarray(inputs["questions"])
    b = facts.shape[0]
    ncores = b // bc

    flens = (np.asarray(inputs["facts_mask"]).reshape(b * nf, l) == 0).sum(-1)
    qlens = (np.asarray(inputs["question_masks"]) == 0).sum(-1)
    assert (flens == l).all() and (qlens == ql).all(), \
        "kernel requires full-length sequences (masks all zero)"

    fx = emb[facts.reshape(-1)].astype(bf16).reshape(b, nf * l, H)
    qx = emb[questions.reshape(-1)].astype(bf16).reshape(b, ql, H)

    ii = {k: np.asarray(vv, np.float32) for k, vv in inputs.items()
          if k not in ("facts", "facts_mask", "questions", "question_masks",
                       "num_decode")}

    def wt_tiles(w, kt):
        wt = w.T.reshape(kt, 128, w.shape[0]).transpose(1, 0, 2)
        return np.ascontiguousarray(wt).reshape(128, kt * w.shape[0]).astype(bf16)

    def col_tiles(x, ncol):
        return np.ascontiguousarray(x.reshape(ncol, 128).T).astype(np.float32)

    shared = {}
    for nm, wih, whh in (("f", "ig_Wih", "ig_Whh"), ("q", "qg_Wih", "qg_Whh"),
                         ("a", "a_Wih", "a_Whh"), ("m", "m_Wih", "m_Whh")):
        shared[f"w_{nm}_ih"] = wt_tiles(ii[wih], HQ)
        shared[f"w_{nm}_hh"] = wt_tiles(ii[whh], HQ)
    shared["w_ans_ih"] = wt_tiles(ii["ans_Wih"], 2 * HQ)
    shared["w_ans_hh"] = wt_tiles(ii["ans_Whh"], HQ)
    g1 = ii["g_w1"].T  # (4H, H)
    shared["g1t"] = np.ascontiguousarray(
        g1.reshape(16, 128, H).transpose(1, 0, 2)).reshape(128, 16 * H).astype(bf16)
    shared["g2t"] = col_tiles(ii["g_w2"][0], HQ).astype(bf16)
    fcw = ii["fc_w"][:v]
    shared["fct"] = np.ascontiguousarray(
        fcw.T.reshape(HQ, 128, v).transpose(1, 0, 2)).astype(bf16)
    shared["fcb"] = ii["fc_b"][:v].reshape(1, v).astype(bf16)
    shared["y0t"] = col_tiles(emb[1], HQ).astype(bf16)

    for nm, bih, bhh in (("f", "ig_bih", "ig_bhh"), ("q", "qg_bih", "qg_bhh"),
                         ("a", "a_bih", "a_bhh"), ("m", "m_bih", "m_bhh"),
                         ("ans", "ans_bih", "ans_bhh")):
        bi, bh = ii[bih], ii[bhh]
        if nm in ("q", "a", "ans"):
            gib = np.concatenate([(bi + bh)[0:2 * H], bi[2 * H:3 * H]])
            shared[f"gib_{nm}"] = col_tiles(gib, MT)
        shared[f"bnhh_{nm}"] = col_tiles(bh[2 * H:3 * H], 4)
        if nm in ("f", "m"):
            shared[f"brz_{nm}"] = col_tiles((bi + bh)[0:2 * H], 8)
            shared[f"bnih_{nm}"] = col_tiles(bi[2 * H:3 * H], 4)
    shared["gb1"] = col_tiles(ii["g_b1"], HQ)
    shared["gb2"] = ii["g_b2"].reshape(1, 1).astype(np.float32)

    in_maps = []
    for c in range(ncores):
        m = dict(shared)
        m["fx"] = np.ascontiguousarray(
            fx[c * bc:(c + 1) * bc].reshape(bc * nf, l, H))
        m["qx"] = np.ascontiguousarray(
            qx[c * bc:(c + 1) * bc].reshape(bc * ql, H))
        in_maps.append(m)
    return in_maps


def kernel(**inputs):
    nd = int(np.asarray(inputs["num_decode"]))
    cfg = Cfg(nd=nd)
    if cfg.key not in _COMPILED:
        _COMPILED[cfg.key] = build(cfg)
    nc = _COMPILED[cfg.key]
    in_maps = host_prep(inputs, cfg)
    res = bass_utils.run_bass_kernel_spmd(nc, in_maps,
                                          core_ids=list(range(N_CORES)))
    out = np.concatenate([res.results[c]["out"] for c in range(N_CORES)], 0)
    return np.ascontiguousarray(out.astype(np.float32))

